# revision 62
# baseline (speedup 1.0000x reference)
"""Trainium2 Bass kernel for NemotronFlash Mamba2 block (optimized).

Full-model shapes: B=2, L=2048, D_MODEL=2048, D_INNER=4096, D_STATE=128,
D_CONV=4, HEADS=64, P=64, CHUNK=128.

Sharding: 8 cores = 2 (batch) x 4 (head-groups of 16 heads).  Each core
computes its batch element end-to-end for its 16 heads / 1024 d_inner
channels.  The gated RMSNorm couples head-groups only through a
per-position sum of squares, so each core emits:
  po    [2048, 2048]    : (W_out_slice * norm_weight) @ yg   (unnormalized)
  ygout [128, 8, 2048]  : the gated y (bf16), from which the host computes
                          ssq[l] = sum over local channels of yg^2
and the host combines:  out[b] = sum_g(po).T * rsqrt(sum_g(ssq)/4096 + eps).

Pipeline structure (single pass, engines overlapped):
  - in_proj xBC tiles first; causal conv (bf16 accum) runs on DVE in-place
    in sx while PE continues; z tiles follow, and G = B^T C matmuls for all
    chunks run on PE interleaved with the z matmuls.
  - chunk transposes (x^T, B^T) are DMA'd on both HWDGE queues, prefetched.
  - SSD chunk loop: DVE does the per-head elementwise work, PE does the
    small matmuls; out_proj is interleaved per 512-column block as soon as
    its 4 chunks of yg are ready, keeping PE busy during the DVE-bound SSD.
  - yg is shipped to the host per 512-col block; the host does the tiny
    sum-of-squares, freeing a PSUM bank so ypsum double-buffers.
"""

import numpy as np

import concourse.bass as bass
import concourse.mybir as mybir
import concourse.tile as tile
from concourse import bacc
from concourse.bass import ds, ts
from concourse.bass_utils import run_bass_kernel_spmd
from concourse.masks import make_identity

FP32 = mybir.dt.float32
BF16 = mybir.dt.bfloat16

# model dims
B_SZ, SEQ, DM = 2, 2048, 2048
D_INNER, D_STATE, D_CONV, HEADS, PDIM, CHUNK = 4096, 128, 4, 64, 64, 128
CONV_DIM = D_INNER + 2 * D_STATE          # 4352
D_IN_PROJ = 2 * D_INNER + 2 * D_STATE + HEADS  # 8512

# per-core dims (4-way head TP)
TPG = 4
HL = HEADS // TPG                 # 16 local heads
DIL = D_INNER // TPG              # 1024 local d_inner channels
NXT = DIL // 128                  # 8 x-channel tiles
NCONVT = NXT + 2                  # + B tile + C tile = 10
NFT = NXT * 2 + 2                 # 18 in_proj F tiles (z, x, B, C)
FPAD = NFT * 128                  # 2304
NKT = DM // 128                   # 16 contraction tiles for in_proj
NCH = SEQ // CHUNK                # 16 chunks
NDMT = DM // 128                  # 16 out rows tiles
LB = 512                          # l-block for 512-wide matmuls
NLB = SEQ // LB                   # 4
PADSEQ = SEQ + 3                  # conv left pad

_CACHE = {}


def _build():
    nc = bacc.Bacc(None, target_bir_lowering=False)

    # ---------------- I/O ----------------
    hsT_d = nc.dram_tensor("hsT", [DM, SEQ], BF16, kind="ExternalInput")
    win_d = nc.dram_tensor("winT", [128, NFT, NKT, 128], BF16, kind="ExternalInput")
    wout_d = nc.dram_tensor("woutT", [DIL, DM], BF16, kind="ExternalInput")
    convw_d = nc.dram_tensor("convw", [128, NCONVT, D_CONV], FP32, kind="ExternalInput")
    convb_d = nc.dram_tensor("convb", [128, NCONVT], FP32, kind="ExternalInput")
    mpre_d = nc.dram_tensor("mpre", [128, NCH, HL, CHUNK], BF16, kind="ExternalInput")
    sdo_d = nc.dram_tensor("sdo_rep", [128, NCH, HL, CHUNK], BF16, kind="ExternalInput")
    dtds_d = nc.dram_tensor("dtds_rep", [128, NCH, HL, PDIM], BF16, kind="ExternalInput")
    cd_d = nc.dram_tensor("cd_rep", [128, NCH, HL], FP32, kind="ExternalInput")
    drep_d = nc.dram_tensor("d_rep", [128, NXT, 128], BF16, kind="ExternalInput")
    po_d = nc.dram_tensor("po", [DM, SEQ], FP32, kind="ExternalOutput")
    yg_d = nc.dram_tensor("ygout", [128, NXT, SEQ], BF16, kind="ExternalOutput")

    with tile.TileContext(nc) as tc:
        with tc.tile_pool(name="const", bufs=1) as cpool, \
             tc.tile_pool(name="persist", bufs=1) as pp:

            # ---------------- constants / small inputs ----------------
            ones_bf = cpool.tile([128, 1], BF16)
            nc.vector.memset(ones_bf, 1.0)
            idn_bf = cpool.tile([128, 128], BF16)
            make_identity(nc, idn_bf)
            convw_sb = cpool.tile([128, NCONVT, D_CONV], FP32)
            convb_sb = cpool.tile([128, NCONVT], FP32)
            cd_sb = cpool.tile([128, NCH, HL], FP32)
            drep_sb = cpool.tile([128, NXT, 128], BF16)

            # ---------------- persistent activations ----------------
            sz_bf = pp.tile([128, NXT, SEQ], BF16)        # silu(z); yg in-place later
            sx_bf = pp.tile([128, NCONVT, PADSEQ], BF16)  # xBC pre-conv then conv+silu
            gm_sb = pp.tile([128, NCH, CHUNK], BF16)      # G = B^T C per chunk
            hrun_bf = pp.tile([128, HL, PDIM], BF16)
            # x^T / B^T for every chunk; filled by DMA transposes streamed
            # during P1 on the sync queue (allocated early so they don't
            # wait on P1's SBUF regions)
            xt_all = pp.tile([128, NCH, NXT + 1, CHUNK], BF16)
            nc.vector.memset(sx_bf[:, :, 0:3], 0.0)

            # ================= P1a: in_proj xBC tiles + conv =================
            p1_ctx = tc.tile_pool(name="p1", bufs=1)
            p1 = p1_ctx.__enter__()
            hsT_sb = p1.tile([128, NKT, SEQ], BF16)
            for ko in range(NKT):
                if ko < 2:  # lb-granular pieces so the first matmuls start early
                    for lb in range(NLB):
                        nc.sync.dma_start(
                            hsT_sb[:, ko, ds(lb * LB, LB)],
                            hsT_d[ts(ko, 128), ds(lb * LB, LB)],
                        )
                else:
                    nc.sync.dma_start(hsT_sb[:, ko, :], hsT_d[ts(ko, 128), :])
            nc.sync.dma_start(convw_sb[:], convw_d[:])
            nc.sync.dma_start(convb_sb[:], convb_d[:])
            nc.sync.dma_start(cd_sb[:], cd_d[:])
            nc.sync.dma_start(drep_sb[:], drep_d[:])

            p1w_ctx = tc.tile_pool(name="p1w", bufs=3)
            p1w = p1w_ctx.__enter__()

            def in_proj_tile(f, psum_pool, evict, split_wf=False):
                wf = p1w.tile([128, NKT, 128], BF16, tag="wf")
                if split_wf:
                    nc.scalar.dma_start(wf[:, 0:2, :], win_d[:, f, 0:2, :])
                    nc.scalar.dma_start(wf[:, 2:NKT, :], win_d[:, f, 2:NKT, :])
                else:
                    nc.scalar.dma_start(wf[:], win_d[:, f, :, :])
                pss = [psum_pool.tile([128, LB], FP32, tag="p1ps",
                                      name=f"p1ps_{f}_{lb}") for lb in range(NLB)]
                for k in range(NKT):
                    for lb in range(NLB):
                        nc.tensor.matmul(
                            pss[lb][:], wf[:, k, :], hsT_sb[:, k, ds(lb * LB, LB)],
                            start=(k == 0), stop=(k == NKT - 1),
                        )
                for lb in range(NLB):
                    evict(lb, pss[lb])

            with tc.tile_pool(name="p1ps_a", bufs=8, space="PSUM") as psa, \
                 tc.tile_pool(name="cacc", bufs=2) as caccp:
                for f in range(NXT, NFT):
                    t = f - NXT

                    def evict_xbc(lb, ps, t=t):
                        nc.scalar.copy(sx_bf[:, t, ds(3 + lb * LB, LB)], ps[:])

                    in_proj_tile(f, psa, evict_xbc, split_wf=(f == NXT))
                    # causal depthwise conv over full seq, bf16 accum, then
                    # silu written back in place over the pre-conv values
                    acc = caccp.tile([128, SEQ], BF16, tag="cacc")
                    nc.vector.tensor_scalar_mul(
                        acc[:], sx_bf[:, t, 0:SEQ], convw_sb[:, t, 0:1],
                    )
                    for k in range(1, D_CONV):
                        nc.vector.scalar_tensor_tensor(
                            acc[:], sx_bf[:, t, ds(k, SEQ)], convw_sb[:, t, k:k + 1],
                            acc[:], mybir.AluOpType.mult, mybir.AluOpType.add,
                        )
                    nc.scalar.activation(
                        sx_bf[:, t, 3:PADSEQ], acc[:],
                        mybir.ActivationFunctionType.Silu,
                        bias=convb_sb[:, t:t + 1],
                    )

            # G psum lives from mid-P1 through the SSD loop
            gpool_ctx = tc.tile_pool(name="gps", bufs=1, space="PSUM")
            gpool = gpool_ctx.__enter__()
            gps = gpool.tile([128, CHUNK], FP32)

            # ---- G = B^T C for every chunk (PE; overlaps z matmuls) ----
            for c in range(NCH):
                cs3 = ds(3 + c * CHUNK, CHUNK)
                nc.tensor.matmul(
                    gps[:], sx_bf[:, NXT, cs3], sx_bf[:, NXT + 1, cs3],
                    start=True, stop=True,
                )
                nc.scalar.copy(gm_sb[:, c, :], gps[:])

            # ================= P1b: in_proj z tiles =================
            # PE transposes of x / B tiles interleave with the z matmuls;
            # evictions to xt_all on ACT
            tjobs = [(c, t) for c in range(NCH) for t in range(NXT + 1)]
            with tc.tile_pool(name="p1ps_z", bufs=5, space="PSUM") as psz, \
                 tc.tile_pool(name="xtps", bufs=2, space="PSUM") as xtpsp:
                for f in range(NXT):

                    def evict_z(lb, ps, f=f):
                        nc.scalar.activation(
                            sz_bf[:, f, ds(lb * LB, LB)], ps[:],
                            mybir.ActivationFunctionType.Silu,
                        )

                    in_proj_tile(f, psz, evict_z)
                    for c, t in tjobs[f * 18:(f + 1) * 18]:
                        xtps = xtpsp.tile([128, CHUNK], BF16, tag="xtps")
                        nc.tensor.transpose(
                            xtps[:], sx_bf[:, t, ds(3 + c * CHUNK, CHUNK)],
                            idn_bf[:],
                        )
                        nc.vector.tensor_copy(xt_all[:, c, t, :], xtps[:])

            p1w_ctx.__exit__(None, None, None)
            p1_ctx.__exit__(None, None, None)
            gpool_ctx.__exit__(None, None, None)

            # ================= P3: chunked SSD + interleaved out_proj =======
            with tc.tile_pool(name="late", bufs=1) as latep, \
                 tc.tile_pool(name="wk", bufs=2) as wk, \
                 tc.tile_pool(name="p4ev", bufs=2) as p4ev, \
                 tc.tile_pool(name="ppy", bufs=2, space="PSUM") as ppy, \
                 tc.tile_pool(name="pps", bufs=1, space="PSUM") as pps, \
                 tc.tile_pool(name="ppo", bufs=2, space="PSUM") as ppo:

                wout_sb = latep.tile([128, NXT, DM], BF16)
                for ko in range(NXT):
                    nc.scalar.dma_start(wout_sb[:, ko, :], wout_d[ts(ko, 128), :])

                # software-pipelined emission: stage A (loads + independent
                # elementwise prep) runs LOOKAHEAD chunks ahead of stage B
                # (matmuls + the serial recurrence + y assembly)
                stageA = {}

                def emit_a(c):
                    cs3 = ds(3 + c * CHUNK, CHUNK)
                    xt = xt_all[:, c]
                    m_all = wk.tile([128, HL, CHUNK], BF16, tag="m_all",
                                    name=f"m_all{c}", bufs=4)
                    nc.sync.dma_start(m_all[:], mpre_d[:, c, :, :])
                    csd = wk.tile([128, HL, CHUNK], BF16, tag="csd",
                                  name=f"csd{c}", bufs=3)
                    nc.sync.dma_start(csd[:], sdo_d[:, c, :, :])
                    ud = None
                    if c < NCH - 1:
                        dtds = wk.tile([128, HL, PDIM], BF16, tag="dtds",
                                       name=f"dtds{c}", bufs=3)
                        nc.sync.dma_start(dtds[:], dtds_d[:, c, :, :])
                        # u' = x^T * (dt * decay_states)
                        ud = wk.tile([128, HL, PDIM], BF16, tag="ud",
                                     name=f"ud{c}", bufs=3)
                        nc.vector.tensor_tensor(
                            ud[:],
                            xt[:, 0:NXT, :].rearrange("p a (h q) -> p (a h) q", h=2),
                            dtds[:], mybir.AluOpType.mult,
                        )
                    # csd = exp(dAcs) * C  (in place)
                    nc.vector.tensor_tensor(
                        csd[:], csd[:],
                        sx_bf[:, NXT + 1:NXT + 2, cs3].to_broadcast(
                            (128, HL, CHUNK)),
                        mybir.AluOpType.mult,
                    )
                    # M = mpre * G  (mpre carries the causal mask zeros)
                    nc.vector.tensor_tensor(
                        m_all[:], m_all[:],
                        gm_sb[:, c:c + 1, :].to_broadcast((128, HL, CHUNK)),
                        mybir.AluOpType.mult,
                    )
                    stageA[c] = (m_all, csd, ud)

                def emit_b(c):
                    cs = ds(c * CHUNK, CHUNK)
                    cs3 = ds(3 + c * CHUNK, CHUNK)
                    xt = xt_all[:, c]
                    m_all, csd, ud = stageA.pop(c)

                    # all-head chunk states in two N=512 matmuls [n, (h p)]
                    if c < NCH - 1:
                        spsum = pps.tile([128, HL, PDIM], FP32, tag="spsum")
                        for g in range(2):
                            nc.tensor.matmul(
                                spsum[:, ds(g * 8, 8), :],
                                xt[:, NXT, :],
                                ud[:, ds(g * 8, 8), :],
                                start=True, stop=True,
                            )

                    # Y_diag (+ Y_off for c>0) accumulated per head
                    ypsum = ppy.tile([128, NXT, CHUNK], FP32, tag="ypsum")
                    for h in range(HL):
                        t, half = h // 2, h % 2
                        yout = ypsum[ds(half * PDIM, PDIM), t, :]
                        nc.tensor.matmul(
                            yout, xt[:, t, ds(half * PDIM, PDIM)],
                            m_all[:, h, :],
                            start=True, stop=(c == 0),
                        )
                        if c > 0:
                            nc.tensor.matmul(
                                yout, hrun_bf[:, h, :], csd[:, h, :],
                                start=False, stop=True,
                            )

                    # inter-chunk recurrence (batched over heads, bf16 state)
                    if c == 0:
                        nc.vector.tensor_copy(hrun_bf[:], spsum[:])
                    elif c < NCH - 1:
                        nc.vector.tensor_tensor(
                            hrun_bf[:], hrun_bf[:],
                            cd_sb[:, c, :, None].to_broadcast((128, HL, PDIM)),
                            mybir.AluOpType.mult,
                        )
                        nc.vector.tensor_tensor(
                            hrun_bf[:], hrun_bf[:], spsum[:],
                            mybir.AluOpType.add,
                        )

                    # evict ypsum via ACT so the bank frees without waiting
                    # on the DVE chain, then assemble y in SBUF at 2x mode:
                    # yg = (yev + D*x) * silu(z), in place in sz
                    yev = wk.tile([128, NXT, CHUNK], BF16, tag="yev")
                    nc.scalar.copy(yev[:], ypsum[:])
                    dx = wk.tile([128, NXT, CHUNK], BF16, tag="dx")
                    nc.vector.tensor_tensor(
                        dx[:], sx_bf[:, 0:NXT, cs3], drep_sb[:],
                        mybir.AluOpType.mult,
                    )
                    nc.vector.tensor_tensor(
                        dx[:], yev[:], dx[:], mybir.AluOpType.add,
                    )
                    nc.vector.tensor_tensor(
                        sz_bf[:, :, cs], dx[:], sz_bf[:, :, cs],
                        mybir.AluOpType.mult,
                    )

                    if c % 4 == 3:
                        lb = c // 4
                        # ship yg for this block; host computes the ssq
                        nc.sync.dma_start(
                            yg_d[:, :, ds(lb * LB, LB)], sz_bf[:, :, ds(lb * LB, LB)],
                        )
                    # out_proj jobs: full 512-col blocks, except the last one
                    # which splits into 2x256 so its first half is ready two
                    # chunks before the final chunk completes
                    if c % 4 == 3 and c < NCH - 3:
                        p4jobs.extend((c // 4 * LB, LB, dm) for dm in range(NDMT))
                    elif c == NCH - 3:
                        p4jobs.extend((3 * LB, 256, dm) for dm in range(NDMT))
                    elif c == NCH - 1:
                        p4jobs.extend((3 * LB + 256, 256, dm) for dm in range(NDMT))

                def emit_p4(col0, w, dm):
                    pso = ppo.tile([128, LB], FP32, tag="pso",
                                   name=f"pso{dm}_{col0}")
                    for k in range(NXT):
                        nc.tensor.matmul(
                            pso[:, 0:w],
                            wout_sb[:, k, ts(dm, 128)],
                            sz_bf[:, k, ds(col0, w)],
                            start=(k == 0), stop=(k == NXT - 1),
                        )
                    ev = p4ev.tile([128, LB], FP32, tag="ev")
                    nc.scalar.copy(ev[:, 0:w], pso[:, 0:w])
                    nc.sync.dma_start(
                        po_d[ts(dm, 128), ds(col0, w)], ev[:, 0:w],
                    )

                # out_proj dm-blocks are drip-fed between chunks so PE has
                # filler while the per-chunk DVE chain runs
                p4jobs = []
                LOOKAHEAD = 2
                for c in range(NCH + LOOKAHEAD):
                    if c < NCH:
                        emit_a(c)
                    if c >= LOOKAHEAD:
                        emit_b(c - LOOKAHEAD)
                        for col0, w, dm in p4jobs[:5]:
                            emit_p4(col0, w, dm)
                        del p4jobs[:5]
                for col0, w, dm in p4jobs:
                    emit_p4(col0, w, dm)

    nc.compile()
    return nc


def _prep_core_inputs(inputs, b, g):
    hs = inputs["hidden_states"]
    W_in, W_conv, b_conv = inputs["W_in"], inputs["W_conv"], inputs["b_conv"]
    A_log, D, dt_bias = inputs["A_log"], inputs["D"], inputs["dt_bias"]
    nw, W_out = inputs["norm_weight"], inputs["W_out"]

    zs = slice(g * DIL, (g + 1) * DIL)
    xs = slice(D_INNER + g * DIL, D_INNER + (g + 1) * DIL)
    bcs = slice(2 * D_INNER, 2 * D_INNER + 2 * D_STATE)
    dts = slice(2 * D_INNER + 2 * D_STATE + g * HL,
                2 * D_INNER + 2 * D_STATE + (g + 1) * HL)
    hsl = slice(g * HL, (g + 1) * HL)

    W_local = np.concatenate([W_in[zs], W_in[xs], W_in[bcs]], axis=0)  # [2304, DM]
    cw = np.concatenate([W_conv[g * DIL:(g + 1) * DIL, 0, :],
                         W_conv[D_INNER:, 0, :]], axis=0)          # [1280, 4]
    cb = np.concatenate([b_conv[g * DIL:(g + 1) * DIL], b_conv[D_INNER:]])  # [1280]

    # dt scalar path on host (tiny): softplus, per-chunk cumsum, derived scalars
    hsb = hs[b].astype(np.float32)
    dt_raw = hsb @ W_in[dts].astype(np.float32).T            # [SEQ, HL]
    dt = np.logaddexp(0.0, dt_raw + dt_bias[hsl][None, :]).astype(np.float32)
    dA = dt * (-np.exp(A_log[hsl]))[None, :]                 # [SEQ, HL]
    dAcs = np.cumsum(dA.reshape(NCH, CHUNK, HL), axis=1,
                     dtype=np.float32)                       # [NCH, CHUNK, HL]
    last = dAcs[:, CHUNK - 1, :]                             # [NCH, HL]
    # dtds[l, c, h] = dt[c,l,h] * exp(last[c,h] - dAcs[c,l,h]); replicate to P
    dtds = (dt.reshape(NCH, CHUNK, HL)
            * np.exp(last[:, None, :] - dAcs)).transpose(1, 0, 2)  # [128, NCH, HL]
    dtds_rep = np.broadcast_to(
        dtds[:, :, :, None], (CHUNK, NCH, HL, PDIM))         # [128, NCH, HL, P]
    cd_rep = np.broadcast_to(np.exp(last)[None], (128, NCH, HL))
    # mpre[s, c, h, l] = exp(dAcs[c,l,h] - dAcs[c,s,h]) * dt[c,s,h] for l>=s
    seg = dAcs[:, None, :, :] - dAcs[:, :, None, :]          # [NCH, s, l, HL]
    np.minimum(seg, 0.0, out=seg)
    np.exp(seg, out=seg)
    seg *= np.tril(np.ones((CHUNK, CHUNK), np.float32)).T[None, :, :, None]
    seg *= dt.reshape(NCH, CHUNK, HL)[:, :, None, :]
    mpre = np.ascontiguousarray(seg.transpose(1, 0, 3, 2))   # [128, NCH, HL, 128]
    # sdo_rep[p, c, h, l] = exp(dAcs[c,l,h]), replicated across partitions
    sdo = np.exp(dAcs).transpose(0, 2, 1)                    # [NCH, HL, CHUNK]
    sdo_rep = np.broadcast_to(sdo[None], (128, NCH, HL, CHUNK))

    import ml_dtypes
    bf = ml_dtypes.bfloat16
    return {
        "hsT": np.ascontiguousarray(hsb.T).astype(bf),
        # [p, f, ko, fq] layout: wf tile loads are one contiguous run per
        # partition (128 descriptors instead of 2048)
        "winT": np.ascontiguousarray(
            W_local.reshape(NFT, 128, NKT, 128).transpose(3, 0, 2, 1)).astype(bf),
        # norm_weight folded into the out projection
        "woutT": np.ascontiguousarray(
            (W_out[:, zs] * nw[zs][None, :]).T).astype(bf),
        "convw": np.ascontiguousarray(
            cw.reshape(NCONVT, 128, D_CONV).transpose(1, 0, 2)).astype(np.float32),
        "convb": np.ascontiguousarray(
            cb.reshape(NCONVT, 128).T).astype(np.float32),
        "mpre": mpre.astype(bf),
        "sdo_rep": np.ascontiguousarray(sdo_rep).astype(bf),
        "dtds_rep": np.ascontiguousarray(dtds_rep).astype(bf),
        "cd_rep": np.ascontiguousarray(cd_rep).astype(np.float32),
        "d_rep": np.ascontiguousarray(np.broadcast_to(
            np.repeat(D[hsl], PDIM).reshape(NXT, 128).T[:, :, None],
            (128, NXT, 128))).astype(bf),
    }


def run(inputs, trace=False):
    import ml_dtypes  # noqa: F401  (ensures bfloat16 dtype is registered)
    if "nc" not in _CACHE:
        _CACHE["nc"] = _build()
    nc = _CACHE["nc"]

    in_maps = []
    for core in range(8):
        b, g = core // TPG, core % TPG
        in_maps.append(_prep_core_inputs(inputs, b, g))
    res = run_bass_kernel_spmd(nc, in_maps, core_ids=list(range(8)), trace=trace)

    out = np.zeros((B_SZ, SEQ, DM), np.float32)
    for b in range(B_SZ):
        po_sum = np.zeros((DM, SEQ), np.float32)
        ssq_sum = np.zeros((SEQ,), np.float32)
        for g in range(TPG):
            r = res.results[b * TPG + g]
            po_sum += r["po"]
            yg = r["ygout"].astype(np.float32)
            ssq_sum += np.einsum("ptl,ptl->l", yg, yg)
        rms = 1.0 / np.sqrt(ssq_sum / D_INNER + 1e-5)
        out[b] = (po_sum * rms[None, :]).T
    return out, res


def kernel(**inputs):
    out, _ = run(inputs, trace=False)
    return out
